# revision 10
# baseline (speedup 1.0000x reference)
"""Trainium2 Bass kernel for Bert-AvgPooling-GCN (ragged sequence).

Strategy (8-core SPMD, one program, per-core data):
- Data-parallel over sentences: core c handles sentences [8c, 8c+8).
- All raggedness is encoded in per-core int32 index tensors consumed by
  indirect DMA, so a single compiled program serves all cores.
- Per-word token max: each word's tokens are CONSECUTIVE rows of
  sequence_output (token_indices = b*S + [0..ntok)), so the k-th token of
  word i lives at row start_i + min(k, t_i - 1). Four duplicate-capped row
  gathers + 3 elementwise maxes produce the padded word tensor
  x[s*120 + p] directly (pad rows are garbage; the zeroed adj columns make
  them irrelevant to valid outputs).
- GCN layer (x3): transpose h -> hT on PE; z = h @ W via float32r matmuls
  (full rate at N>=256); y = adjT_scaled @ z where adjT_scaled is
  transpose(adj / denom) computed once per sentence; h' = relu(y + b).
- Words scattered compactly (global word order) to DRAM scratch; clauses
  (5 consecutive global words) via 5 duplicate-capped gathers + max; FC to
  16 logits on device. Clauses split across a core boundary are exported
  as partial-max rows and combined on the host (<= 7 rows of 1180).
"""

import sys
from contextlib import ExitStack
import numpy as np

sys.path.insert(0, "/opt/trn_rl_repo")

B, S, D = 64, 512, 768
CLAUSE = 5
NCORES = 8
SPB = B // NCORES          # sentences per core
MW = 120                   # max words per sentence
WC = SPB * MW              # padded word slots per core (960)
CMAX = 160                 # padded clauses per core (two halves of 80)
CH = CMAX // 2
TRASH = WC                 # trash row in wordsH scratch
OOB = 7                    # out-of-bounds marker for cbnd scatter (bound=1)

MM_DT = "float32r"         # matmul dtype for the big GEMMs


def _build_program():
    import concourse.bass as bass
    import concourse.tile as tile
    from concourse import mybir
    from concourse.masks import make_identity

    f32 = mybir.dt.float32
    i32 = mybir.dt.int32
    mmdt = getattr(mybir.dt, MM_DT)
    Relu = mybir.ActivationFunctionType.Relu
    Copy = mybir.ActivationFunctionType.Copy

    nc = bass.Bass()

    seqc = nc.declare_dram_parameter("seqc", [SPB * S, D], f32, isOutput=False)
    adjc = nc.declare_dram_parameter("adjc", [SPB, MW, MW], f32, isOutput=False)
    Ws = [nc.declare_dram_parameter(f"W{l}", [D, D], f32, isOutput=False)
          for l in range(3)]
    bs = [nc.declare_dram_parameter(f"b{l}", [D], f32, isOutput=False)
          for l in range(3)]
    wfc = nc.declare_dram_parameter("wfc", [D, 16], f32, isOutput=False)
    bfc = nc.declare_dram_parameter("bfc", [16], f32, isOutput=False)
    gi = nc.declare_dram_parameter("gi", [WC, 4], i32, isOutput=False)
    sc = nc.declare_dram_parameter("sc", [WC, 1], i32, isOutput=False)
    cg = nc.declare_dram_parameter("cg", [CMAX, 5], i32, isOutput=False)
    bidx = nc.declare_dram_parameter("bidx", [CMAX, 1], i32, isOutput=False)

    logits = nc.declare_dram_parameter("logits", [CMAX, 16], f32, isOutput=True)
    cbnd = nc.declare_dram_parameter("cbnd", [2, D], f32, isOutput=True)

    wordsH = nc.dram_tensor("wordsH", [WC + 1, D], f32)

    KC = D // 128  # 6 contraction chunks

    with tile.TileContext(nc) as tc, ExitStack() as ctx:
        const = ctx.enter_context(tc.tile_pool(name="const", bufs=1))
        ipool = ctx.enter_context(tc.tile_pool(name="ipool", bufs=3))
        gpool = ctx.enter_context(tc.tile_pool(name="gpool", bufs=6))
        xpool = ctx.enter_context(tc.tile_pool(name="xpool", bufs=2))
        apool = ctx.enter_context(tc.tile_pool(name="apool", bufs=2))
        tpool = ctx.enter_context(tc.tile_pool(name="tpool", bufs=2))
        zpool = ctx.enter_context(tc.tile_pool(name="zpool", bufs=2))
        hpool = ctx.enter_context(tc.tile_pool(name="hpool", bufs=2))
        cpool = ctx.enter_context(tc.tile_pool(name="cpool", bufs=6))
        lpool = ctx.enter_context(tc.tile_pool(name="lpool", bufs=2))
        wstg_pool = ctx.enter_context(tc.tile_pool(name="wstg", bufs=1))
        tpsum = ctx.enter_context(tc.tile_pool(name="tpsum", bufs=2, space="PSUM"))
        apsum = ctx.enter_context(tc.tile_pool(name="apsum", bufs=1, space="PSUM"))
        zpsum = ctx.enter_context(tc.tile_pool(name="zpsum", bufs=2, space="PSUM"))
        ypsum = ctx.enter_context(tc.tile_pool(name="ypsum", bufs=2, space="PSUM"))

        ident = const.tile([128, 128], f32)
        make_identity(nc, ident[:])

        # Weights as [128, KC, D] (partition = K % 128, chunk = K // 128)
        W_sb = []
        for l in range(3):
            wstg = wstg_pool.tile([128, KC, D], f32, tag="wstage")
            for c in range(KC):
                eng = nc.sync if c % 2 == 0 else nc.scalar
                eng.dma_start(out=wstg[:, c, :], in_=Ws[l][c * 128:(c + 1) * 128, :])
            wt = const.tile([128, KC, D], mmdt, tag=f"W{l}")
            nc.vector.tensor_copy(out=wt[:], in_=wstg[:])
            W_sb.append(wt)
        wfc_sb = const.tile([128, KC, 16], f32)
        nc.sync.dma_start(out=wfc_sb[:], in_=wfc.rearrange("(ko p) n -> p ko n", p=128))

        # Bias broadcast tiles [128, D]: DMA with 0-stride partition broadcast
        bb = []
        for l in range(3):
            bt = const.tile([128, D], f32, tag=f"bb{l}")
            nc.sync.dma_start(out=bt[:], in_=bs[l][None, :].to_broadcast([128, D]))
            bb.append(bt)
        bfc_b = const.tile([128, 16], f32)
        nc.sync.dma_start(out=bfc_b[:], in_=bfc[None, :].to_broadcast([128, 16]))

        for s in range(SPB):
            wsl = slice(s * MW, (s + 1) * MW)
            idx_t = ipool.tile([MW, 4], i32, tag="gidx")
            nc.sync.dma_start(out=idx_t[:], in_=gi[wsl, :])
            sc_t = ipool.tile([MW, 1], i32, tag="scidx")
            nc.sync.dma_start(out=sc_t[:], in_=sc[wsl, :])

            g = []
            for k in range(4):
                gk = gpool.tile([MW, D], f32, tag="g")
                nc.gpsimd.indirect_dma_start(
                    out=gk[:], out_offset=None, in_=seqc[:],
                    in_offset=bass.IndirectOffsetOnAxis(ap=idx_t[:, k:k + 1], axis=0))
                g.append(gk)
            x = xpool.tile([MW, D], f32, tag="x")
            nc.vector.tensor_max(out=x[:], in0=g[0][:], in1=g[1][:])
            nc.vector.tensor_max(out=g[2][:], in0=g[2][:], in1=g[3][:])
            nc.vector.tensor_max(out=x[:], in0=x[:], in1=g[2][:])

            # adj prep: adjT_scaled = transpose(adj / (rowsum+1))
            adj_t = apool.tile([MW, MW], f32, tag="adj")
            nc.sync.dma_start(out=adj_t[:], in_=adjc[s])
            dsum = apool.tile([MW, 1], f32, tag="dsum")
            nc.vector.tensor_reduce(out=dsum[:], in_=adj_t[:],
                                    axis=mybir.AxisListType.X, op=mybir.AluOpType.add)
            nc.vector.tensor_scalar_add(dsum[:], dsum[:], 1.0)
            rec = apool.tile([MW, 1], f32, tag="rec")
            nc.vector.reciprocal(rec[:], dsum[:])
            adj_sc = apool.tile([MW, MW], f32, tag="adjsc")
            nc.scalar.activation(out=adj_sc[:], in_=adj_t[:], func=Copy,
                                 scale=rec[:, :1])
            aT_ps = apsum.tile([MW, MW], f32, tag="aT")
            nc.tensor.transpose(out=aT_ps[:], in_=adj_sc[:], identity=ident[:MW, :MW])
            adjT = apool.tile([MW, MW], mmdt, tag="adjT")
            nc.any.tensor_copy(out=adjT[:], in_=aT_ps[:])

            h = x
            for l in range(3):
                hT = tpool.tile([128, KC, MW], mmdt, tag="hT")
                for c in range(KC):
                    tp = tpsum.tile([128, MW], f32, tag="tp")
                    nc.tensor.transpose(out=tp[:], in_=h[:, c * 128:(c + 1) * 128],
                                        identity=ident[:MW, :MW])
                    nc.any.tensor_copy(out=hT[:, c, :], in_=tp[:])
                z_sb = zpool.tile([MW, D], mmdt, tag="z")
                for half in range(2):
                    nsl = slice(half * 384, (half + 1) * 384)
                    zp = zpsum.tile([MW, 384], f32, tag="zp")
                    for c in range(KC):
                        nc.tensor.matmul(
                            out=zp[:], lhsT=hT[:, c, :],
                            rhs=W_sb[l][:, c, nsl],
                            start=(c == 0), stop=(c == KC - 1))
                    nc.any.tensor_copy(out=z_sb[:, nsl], in_=zp[:])
                hn = hpool.tile([MW, D], f32, tag="h")
                for half in range(2):
                    nsl = slice(half * 384, (half + 1) * 384)
                    yp = ypsum.tile([MW, 384], f32, tag="yp")
                    nc.tensor.matmul(out=yp[:], lhsT=adjT[:],
                                     rhs=z_sb[:, nsl],
                                     start=True, stop=True)
                    nc.vector.tensor_add(out=hn[:, nsl], in0=yp[:],
                                         in1=bb[l][:MW, nsl])
                nc.scalar.activation(out=hn[:], in_=hn[:], func=Relu)
                h = hn

            nc.gpsimd.indirect_dma_start(
                out=wordsH[:], out_offset=bass.IndirectOffsetOnAxis(ap=sc_t[:, :1], axis=0),
                in_=h[:], in_offset=None)

        # clause stage: two halves of 76 clauses
        for half in range(2):
            csl = slice(half * CH, (half + 1) * CH)
            cidx = ipool.tile([CH, 5], i32, tag="cidx")
            nc.sync.dma_start(out=cidx[:], in_=cg[csl, :])
            bidx_t = ipool.tile([CH, 1], i32, tag="bidx")
            nc.sync.dma_start(out=bidx_t[:], in_=bidx[csl, :])

            cgt = []
            for j in range(5):
                cj = cpool.tile([CH, D], f32, tag="cg")
                nc.gpsimd.indirect_dma_start(
                    out=cj[:], out_offset=None, in_=wordsH[:],
                    in_offset=bass.IndirectOffsetOnAxis(ap=cidx[:, j:j + 1], axis=0))
                cgt.append(cj)
            cm = cpool.tile([CH, D], f32, tag="cm")
            nc.vector.tensor_max(out=cm[:], in0=cgt[0][:], in1=cgt[1][:])
            nc.vector.tensor_max(out=cgt[2][:], in0=cgt[2][:], in1=cgt[3][:])
            nc.vector.tensor_max(out=cm[:], in0=cm[:], in1=cgt[2][:])
            nc.vector.tensor_max(out=cm[:], in0=cm[:], in1=cgt[4][:])

            # export boundary clause partial-max rows (bounds-checked scatter)
            nc.gpsimd.indirect_dma_start(
                out=cbnd[:], out_offset=bass.IndirectOffsetOnAxis(ap=bidx_t[:, :1], axis=0),
                in_=cm[:], in_offset=None,
                bounds_check=1, oob_is_err=False)

            cT = tpool.tile([128, KC, CH], f32, tag="cT")
            for c in range(KC):
                tpf = tpsum.tile([128, MW], f32, tag="tp", name="tpf")
                tp = tpf[:, :CH]
                nc.tensor.transpose(out=tp[:], in_=cm[:, c * 128:(c + 1) * 128],
                                    identity=ident[:CH, :CH])
                nc.any.tensor_copy(out=cT[:, c, :], in_=tp[:])
            lpf = ypsum.tile([MW, 384], f32, tag="yp", name="lpf")
            lp = lpf[:CH, :16]
            for c in range(KC):
                nc.tensor.matmul(out=lp[:], lhsT=cT[:, c, :], rhs=wfc_sb[:, c, :],
                                 start=(c == 0), stop=(c == KC - 1))
            lg = lpool.tile([CH, 16], f32, tag="lg")
            nc.vector.tensor_add(out=lg[:], in0=lp[:], in1=bfc_b[:CH, :])
            nc.sync.dma_start(out=logits[csl, :], in_=lg[:])

    _split_waits(nc, cap=1)
    return nc


def _split_waits(nc, cap=1):
    """Walrus in this toolchain rejects instructions carrying more than ~4
    semaphore waits. Split excess waits onto same-engine EventSemaphore nops
    inserted just before the instruction (engines process waits in program
    order, so this preserves the dependency semantics)."""
    from concourse import mybir
    ctr = 0
    for fn in nc.m.functions:
        for bb in fn.blocks:
            il = bb.instructions
            out = []
            changed = False
            for ins in il:
                si = ins.sync_info
                if si is not None and len(si.on_wait) > cap:
                    waits = list(si.on_wait)
                    head, tail = waits[:-cap], waits[-cap:]
                    for i in range(0, len(head), cap):
                        ctr += 1
                        out.append(mybir.InstEventSemaphore(
                            name=f"wsplit-{ctr}", engine=ins.engine, ins=[], outs=[],
                            sync_info=mybir.SyncInfo(on_wait=head[i:i + cap],
                                                     on_update=[])))
                    ins.sync_info = mybir.SyncInfo(on_wait=tail,
                                                   on_update=list(si.on_update))
                    changed = True
                out.append(ins)
            if changed:
                bb.instructions = out
    return nc


def _plan(token_indices, word_seg, word2sent, clause_seg, n_clauses):
    """Host-side per-core index planning from the runtime ragged arrays."""
    token_indices = np.asarray(token_indices)
    word_seg = np.asarray(word_seg)
    word2sent = np.asarray(word2sent)

    Wt = word2sent.shape[0]
    w = np.bincount(word2sent, minlength=B)              # words per sentence
    sent_wstart = np.zeros(B + 1, np.int64)
    sent_wstart[1:] = np.cumsum(w)
    t = np.bincount(word_seg, minlength=Wt)              # tokens per word
    wt_start = np.zeros(Wt + 1, np.int64)
    wt_start[1:] = np.cumsum(t)

    # DRAM row (global) of the k-th token of each word, capped at the last
    rowidx = np.empty((Wt, 4), np.int64)
    for k in range(4):
        rowidx[:, k] = token_indices[wt_start[:-1] + np.minimum(k, t - 1)]

    cores = []
    for c in range(NCORES):
        b0 = c * SPB
        ws = int(sent_wstart[b0])
        we = int(sent_wstart[b0 + SPB])
        wc = we - ws
        assert wc <= WC

        gi = np.zeros((WC, 4), np.int32)
        scv = np.full((WC, 1), TRASH, np.int32)
        for ls in range(SPB):
            b = b0 + ls
            nw = int(w[b])
            gw = np.arange(sent_wstart[b], sent_wstart[b] + nw)
            slot = ls * MW + np.arange(nw)
            gi[slot] = rowidx[gw] - b0 * S
            scv[slot, 0] = gw - ws
        assert gi.min() >= 0 and gi.max() < SPB * S

        fc = ws // CLAUSE
        lc = (we - 1) // CLAUSE
        ncl = lc - fc + 1
        assert ncl <= CMAX and ncl - 1 >= CH

        cgv = np.zeros((CMAX, 5), np.int32)
        q = np.arange(ncl)
        for j in range(5):
            g = CLAUSE * (fc + q) + j
            cgv[:ncl, j] = np.clip(g, ws, we - 1) - ws

        bv = np.full((CMAX, 1), OOB, np.int32)
        bv[0, 0] = 0          # first clause partial row -> cbnd[0]
        bv[ncl - 1, 0] = 1    # last clause partial row -> cbnd[1]

        cores.append(dict(b0=b0, ws=ws, we=we, fc=fc, lc=lc, ncl=ncl,
                          gi=gi, sc=scv, cg=cgv, bidx=bv))
    return cores


_CACHED = {}


def kernel(sequence_output, adj, W1, b1, W2, b2, W3, b3, Wfc, bfc,
           token_indices, word_seg, word2sent, word_pos, clause_seg, n_clauses):
    from concourse.bass_utils import run_bass_kernel_spmd

    sequence_output = np.asarray(sequence_output, dtype=np.float32)
    adj = np.asarray(adj, dtype=np.float32)
    weights = dict(W0=np.asarray(W1, np.float32), W1=np.asarray(W2, np.float32),
                   W2=np.asarray(W3, np.float32))
    biases = dict(b0=np.asarray(b1, np.float32), b1=np.asarray(b2, np.float32),
                  b2=np.asarray(b3, np.float32))
    wfc = np.asarray(Wfc, np.float32)
    bfcv = np.asarray(bfc, np.float32)
    n_clauses = int(n_clauses)

    cores = _plan(token_indices, word_seg, word2sent, clause_seg, n_clauses)

    if "nc" not in _CACHED:
        _CACHED["nc"] = _build_program()
    nc = _CACHED["nc"]

    in_maps = []
    for c, pl in enumerate(cores):
        b0 = pl["b0"]
        m = {
            "seqc": sequence_output[b0:b0 + SPB].reshape(SPB * S, D),
            "adjc": adj[b0:b0 + SPB],
            "wfc": wfc, "bfc": bfcv,
            "gi": pl["gi"], "sc": pl["sc"], "cg": pl["cg"], "bidx": pl["bidx"],
        }
        for l in range(3):
            m[f"W{l}"] = weights[f"W{l}"]
            m[f"b{l}"] = biases[f"b{l}"]
        in_maps.append(m)

    res = run_bass_kernel_spmd(nc, in_maps, core_ids=list(range(NCORES)))
    _CACHED["last_exec_time_ns"] = res.exec_time_ns

    out = np.zeros((n_clauses, 16), np.float32)
    for c, pl in enumerate(cores):
        out[pl["fc"]:pl["lc"] + 1] = res.results[c]["logits"][:pl["ncl"]]

    # fix up clauses split across core boundaries
    for c in range(NCORES - 1):
        we = cores[c]["we"]
        if we % CLAUSE != 0:
            cid = we // CLAUSE
            m = np.maximum(res.results[c]["cbnd"][1],
                           res.results[c + 1]["cbnd"][0])
            out[cid] = m @ wfc + bfcv
    return out
